# revision 19
# baseline (speedup 1.0000x reference)
"""Chamfer distance loss on 8 Trainium2 NeuronCores.

Full inputs: points1 [16, 4096, 3], points2 [16, 4096, 3] (fp32).
Output: scalar fp32 loss = (sum(min_m dist) + sum(min_n dist)) / B.

Sharding: data-parallel over batch B=16 -> 2 batches per core on 8 cores.
Each core computes a partial scalar (sum of row-mins + col-mins for its
batches); host sums the 8 partials and divides by B.

Per-batch device algorithm (per core), v3:
  dist[n, m] = |a_n|^2 + |b_m|^2 - 2 a.b  computed as:
    psum = matmul(lhsT=[ax,ay,az,-.5,-.5,-.5], rhs=[bx,by,bz,bx^2,by^2,bz^2])
         = a.b - |b|^2/2                       (K=6, fp32r, N=512 per bank)
    dist16 = ScalarE Identity((-2)*psum + bias)  bias = |a_n|^2 per partition
  All matmul operands (including the replicated row groups for PE
  tile_position concurrency) and the |a|^2 bias columns are PRECOMPUTED ON
  HOST and DMA'd in directly - no device-side staging/squaring.
  Stripes (128 rows of n) are processed in QUADS of 4; the bf16 dist tiles
  of a quad live in one ring tile [128, 4, 4096] so the row-min fold tree
  runs as ONE DVE op per level over all 4 stripes ([128, 4, w] 3D APs) -
  DVE per-op overhead dominated v1. DVE ops stay <= 2048 elems/partition
  wide (wider flat ops hit a slow path).
  col-min: DVE tensor_tensor min into acc per stripe; final col-min across
  partitions via PE transpose + strided reduce-min; row+col sums via one
  merged reduce-add + one matmul with ones.
"""

import time

import numpy as np

import concourse.bacc as bacc
import concourse.mybir as mybir
import concourse.tile as tile
from concourse import bass_utils
from concourse.masks import make_identity

N_CORES = 8

f32 = mybir.dt.float32
f32r = mybir.dt.float32r
f16 = mybir.dt.bfloat16
AF = mybir.ActivationFunctionType
ALU = mybir.AluOpType
AX = mybir.AxisListType

_CACHE = {}
last_exec_seconds = None  # wall time of the device dispatch (set per call)

QUAD = 4         # stripes per quad (ring depth)
PSW = 2048       # psum group width (2048 | 4096)
PS_BUFS = 2      # psum pool bufs (PSW//512 banks each; total <= 8 banks)
RG = 4           # PE row-groups for concurrent matmuls (1 | 2 | 4)
EVAC_ON = True   # timing attribution: ScalarE evacuation
ROWMIN_ON = True  # timing attribution: t01 + quad fold tree
COLMIN_ON = True  # timing attribution: colacc TTs

NROWS = 32 * (RG - 1) + 6


def _build(bl: int, n: int, m: int, repeat: int = 1):
    """Build the SPMD module for bl batches of [n x 3] vs [m x 3] points.

    repeat > 1 wraps the whole computation in a hardware For_i loop that
    recomputes the same result `repeat` times — used only for timing.
    """
    assert n % (128 * QUAD) == 0 and m % PSW == 0
    n_stripes = n // 128
    n_quads = n_stripes // QUAD
    n_groups = m // PSW

    nc = bacc.Bacc("TRN2", target_bir_lowering=False, debug=False)
    a6d = nc.dram_tensor("a6d", [bl, NROWS, n], f32r, kind="ExternalInput")
    b6d = nc.dram_tensor("b6d", [bl, NROWS, m], f32r, kind="ExternalInput")
    a2d = nc.dram_tensor("a2d", [bl, 128, n // 128], f32, kind="ExternalInput")
    out = nc.dram_tensor("out", [1, bl], f32, kind="ExternalOutput")

    with tile.TileContext(nc) as tc:
        with (
            tc.tile_pool(name="const", bufs=1) as constp,
            tc.tile_pool(name="pts", bufs=2) as ptsp,
            tc.tile_pool(name="acc", bufs=2) as accp,
            tc.tile_pool(name="ring", bufs=2) as ringp,
            tc.tile_pool(name="t01", bufs=1) as t01p,
            tc.tile_pool(name="small", bufs=4) as smallp,
            tc.tile_pool(name="psum", bufs=PS_BUFS, space="PSUM") as psump,
        ):
            ident = constp.tile([128, 128], f16)
            make_identity(nc, ident[:])
            ones128 = constp.tile([128, 1], f32)
            nc.gpsimd.memset(ones128[:], 1.0)
            out_sb = constp.tile([1, bl], f32)

            import contextlib
            loop_ctx = (
                tc.For_i(0, repeat, 1) if repeat > 1 else contextlib.nullcontext()
            )
            with loop_ctx:
                for b in range(bl):
                    a6 = ptsp.tile([NROWS, n], f32r, tag="a6")
                    b6 = ptsp.tile([NROWS, m], f32r, tag="b6")
                    a2c = smallp.tile([128, n_stripes], f32, tag="a2c")
                    nc.sync.dma_start(a6[:], a6d.ap()[b])
                    nc.sync.dma_start(b6[:], b6d.ap()[b])
                    nc.sync.dma_start(a2c[:], a2d.ap()[b])

                    acc = accp.tile([128, m], f16, tag="acc")
                    # mins: cols 0:n_stripes = per-stripe row-mins,
                    #       cols n_stripes:n_stripes+m//128 = col-min blocks
                    mins = smallp.tile([128, n_stripes + m // 128], f16, tag="mins")

                    for q in range(n_quads):
                        ring = ringp.tile([128, QUAD, m], f16, tag="ring")
                        t01 = t01p.tile([128, QUAD, m // 2], f16, tag="t01")
                        for si in range(QUAD):
                            s = q * QUAD + si
                            ssl = slice(128 * s, 128 * (s + 1))
                            for g in range(n_groups):
                                ps = psump.tile([128, PSW], f32, tag="mm")
                                for j in range(PSW // 512):
                                    mo = PSW * g + 512 * j
                                    ro = 32 * ((g * (PSW // 512) + j) % RG)
                                    nc.tensor.matmul(
                                        ps[:, 512 * j : 512 * (j + 1)],
                                        a6[ro : ro + 6, ssl],
                                        b6[ro : ro + 6, mo : mo + 512],
                                        start=True,
                                        stop=True,
                                        tile_position=(ro, 0),
                                    )
                                gsl = slice(PSW * g, PSW * (g + 1))
                                if EVAC_ON:
                                    nc.scalar.activation(
                                        ring[:, si, gsl], ps[:], AF.Identity,
                                        bias=a2c[:, s : s + 1], scale=-2.0,
                                    )
                                else:
                                    nc.vector.memset(ring[:, si, gsl], 1.0)
                            # col-min accumulate, 2048-wide chunks (wider
                            # flat DVE ops hit a slow path)
                            if COLMIN_ON:
                                for c in range(m // 2048):
                                    csl = slice(2048 * c, 2048 * (c + 1))
                                    if s == 0:
                                        nc.vector.tensor_copy(
                                            acc[:, csl], ring[:, si, csl]
                                        )
                                    else:
                                        nc.vector.tensor_tensor(
                                            acc[:, csl], acc[:, csl],
                                            ring[:, si, csl], ALU.min,
                                        )
                            # per-stripe first fold: m -> m/2 (2048-out op)
                            if ROWMIN_ON:
                                nc.vector.tensor_tensor(
                                    t01[:, si, :], ring[:, si, 0 : m // 2],
                                    ring[:, si, m // 2 : m], ALU.min,
                                )
                        # quad-batched fold tree: one op per level, 4 stripes
                        if ROWMIN_ON:
                            w = m // 4
                            while w >= 128:
                                nc.vector.tensor_tensor(
                                    t01[:, :, 0:w], t01[:, :, 0:w],
                                    t01[:, :, w : 2 * w], ALU.min,
                                )
                                w //= 2
                            nc.vector.tensor_reduce(
                                mins[:, q * QUAD : (q + 1) * QUAD],
                                t01[:, :, 0:128],
                                axis=AX.X,
                                op=ALU.min,
                            )
                        else:
                            nc.vector.memset(
                                mins[:, q * QUAD : (q + 1) * QUAD], 0.0
                            )

                    # col-min across partitions: 16 transposes per psum tile,
                    # then one strided reduce-min per psum tile.
                    n_blocks = m // 128
                    if COLMIN_ON:
                        tpb = PSW // 128
                        for k0 in range(0, n_blocks, tpb):
                            pst = psump.tile([128, PSW], f16, tag="mm")
                            kk = min(tpb, n_blocks - k0)
                            for k in range(kk):
                                nc.tensor.transpose(
                                    pst[:, 128 * k : 128 * (k + 1)],
                                    acc[:, 128 * (k0 + k) : 128 * (k0 + k + 1)],
                                    ident[:],
                                )
                            nc.vector.tensor_reduce(
                                mins[:, n_stripes + k0 : n_stripes + k0 + kk],
                                pst[:, 0 : 128 * kk].rearrange(
                                    "p (k x) -> p k x", x=128
                                ),
                                axis=AX.X,
                                op=ALU.min,
                            )
                    else:
                        nc.vector.memset(mins[:, n_stripes:], 0.0)

                    # single merged sum: reduce-add all row-mins and col-mins
                    # then one ones-matmul to collapse partitions
                    tsum = smallp.tile([128, 1], f32, tag="tsum")
                    nc.vector.tensor_reduce(tsum[:], mins[:], axis=AX.X, op=ALU.add)
                    sc = psump.tile([128, PSW], f32, tag="mm")
                    nc.tensor.matmul(
                        sc[0:1, 0:1], tsum[:], ones128[:], start=True, stop=True
                    )
                    nc.vector.tensor_copy(out_sb[0:1, b : b + 1], sc[0:1, 0:1])

                nc.sync.dma_start(out.ap(), out_sb[:])

    nc.finalize()
    return nc


def _prep(points, bl):
    """Host-side: [B, N, 3] fp32 -> per-core lhsT/rhs arrays + |a|^2 bias.

    Returns (x6 [B, NROWS, N], x2c [B, 128, N//128]) where x6 rows
    32*rg + (0..5) = [x, y, z, -0.5, -0.5, -0.5] replicated for each PE
    row-group, and rhs rows 3..5 hold the squared coords instead of -0.5
    (the b-side). The caller picks which rows matter.
    """
    B, N, _ = points.shape
    xT = points.transpose(0, 2, 1)  # [B, 3, N]
    x6 = np.zeros((B, NROWS, N), dtype=np.float32)
    sq = xT * xT
    x2 = sq.sum(axis=1)  # [B, N]
    for rg in range(RG):
        r = 32 * rg
        x6[:, r : r + 3] = xT
    x2c = np.ascontiguousarray(
        x2.reshape(B, N // 128, 128).transpose(0, 2, 1)
    )  # [B, 128, N//128], x2c[b, p, s] = |x_{128 s + p}|^2
    return x6, sq, x2c


def _in_maps(points1, points2):
    points1 = np.ascontiguousarray(np.asarray(points1), dtype=np.float32)
    points2 = np.ascontiguousarray(np.asarray(points2), dtype=np.float32)
    btot = points1.shape[0]
    bl = btot // N_CORES
    a6, _, a2c = _prep(points1, bl)
    b6, bsq, _ = _prep(points2, bl)
    # a-side rows 3:5 = -0.5 consts; b-side rows 3:5 = squared coords
    for rg in range(RG):
        r = 32 * rg
        a6[:, r + 3 : r + 6] = -0.5
        b6[:, r + 3 : r + 6] = bsq
    return [
        {
            "a6d": a6[c * bl : (c + 1) * bl],
            "b6d": b6[c * bl : (c + 1) * bl],
            "a2d": a2c[c * bl : (c + 1) * bl],
        }
        for c in range(N_CORES)
    ]


def kernel(points1, points2):
    global last_exec_seconds
    points1 = np.ascontiguousarray(np.asarray(points1), dtype=np.float32)
    points2 = np.ascontiguousarray(np.asarray(points2), dtype=np.float32)
    btot, n, _ = points1.shape
    m = points2.shape[1]
    bl = btot // N_CORES

    key = (bl, n, m)
    if _CACHE.get("key") != key:
        _CACHE["nc"] = _build(bl, n, m)
        _CACHE["key"] = key
    nc = _CACHE["nc"]

    in_maps = _in_maps(points1, points2)
    t0 = time.time()
    res = bass_utils.run_bass_kernel_spmd(
        nc, in_maps, core_ids=list(range(N_CORES))
    )
    last_exec_seconds = time.time() - t0

    total = np.float64(0.0)
    for r in res.results:
        total += r["out"].astype(np.float64).sum()
    return np.float32(total / btot)


# revision 23
# speedup vs baseline: 1.4961x; 1.4961x over previous
"""Chamfer distance loss on 8 Trainium2 NeuronCores.

Full inputs: points1 [16, 4096, 3], points2 [16, 4096, 3] (fp32).
Output: scalar fp32 loss = (sum(min_m dist) + sum(min_n dist)) / B.

Sharding: data-parallel over batch B=16 -> 2 batches per core on 8 cores.
Each core computes a partial scalar (sum of row-mins + col-mins for its
batches); host sums the 8 partials and divides by B.

Per-batch device algorithm (per core), v3:
  dist[n, m] = |a_n|^2 + |b_m|^2 - 2 a.b  computed as:
    psum = matmul(lhsT=[ax,ay,az,-.5,-.5,-.5], rhs=[bx,by,bz,bx^2,by^2,bz^2])
         = a.b - |b|^2/2                       (K=6, fp32r, N=512 per bank)
    dist16 = ScalarE Identity((-2)*psum + bias)  bias = |a_n|^2 per partition
  All matmul operands (including the replicated row groups for PE
  tile_position concurrency) and the |a|^2 bias columns are PRECOMPUTED ON
  HOST and DMA'd in directly - no device-side staging/squaring.
  Stripes (128 rows of n) are processed in QUADS of 4; the bf16 dist tiles
  of a quad live in one ring tile [128, 4, 4096] so the row-min fold tree
  runs as ONE DVE op per level over all 4 stripes ([128, 4, w] 3D APs) -
  DVE per-op overhead dominated v1. DVE ops stay <= 2048 elems/partition
  wide (wider flat ops hit a slow path).
  col-min: DVE tensor_tensor min into acc per stripe; final col-min across
  partitions via PE transpose + strided reduce-min; row+col sums via one
  merged reduce-add + one matmul with ones.
"""

import time

import numpy as np

import concourse.bacc as bacc
import concourse.mybir as mybir
import concourse.tile as tile
from concourse import bass_utils
from concourse.masks import make_identity

N_CORES = 8

f32 = mybir.dt.float32
f32r = mybir.dt.float32r
f16 = mybir.dt.bfloat16
AF = mybir.ActivationFunctionType
ALU = mybir.AluOpType
AX = mybir.AxisListType

_CACHE = {}
last_exec_seconds = None  # wall time of the device dispatch (set per call)

QUAD = 4         # stripes per quad (ring depth)
PSW = 2048       # psum group width (2048 | 4096)
PS_BUFS = 2      # psum pool bufs (PSW//512 banks each; total <= 8 banks)
RG = 4           # PE row-groups for concurrent matmuls (1 | 2 | 4)
EVAC_ON = True   # timing attribution: ScalarE evacuation
ROWMIN_ON = True  # timing attribution: t01 + quad fold tree
COLMIN_ON = True  # timing attribution: colacc TTs

NROWS = 32 * (RG - 1) + 6


def _build(bl: int, n: int, m: int, repeat: int = 1):
    """Build the SPMD module for bl batches of [n x 3] vs [m x 3] points.

    repeat > 1 wraps the whole computation in a hardware For_i loop that
    recomputes the same result `repeat` times — used only for timing.
    """
    assert n % (128 * QUAD) == 0 and m % PSW == 0
    n_stripes = n // 128
    n_quads = n_stripes // QUAD
    n_groups = m // PSW

    nc = bacc.Bacc("TRN2", target_bir_lowering=False, debug=False)
    a6d = nc.dram_tensor("a6d", [bl, 6, n], f32r, kind="ExternalInput")
    b6d = nc.dram_tensor("b6d", [bl, 6, m], f32r, kind="ExternalInput")
    a2d = nc.dram_tensor("a2d", [bl, 128, n // 128], f32, kind="ExternalInput")
    out = nc.dram_tensor("out", [1, bl], f32, kind="ExternalOutput")

    with tile.TileContext(nc) as tc:
        with (
            tc.tile_pool(name="const", bufs=1) as constp,
            tc.tile_pool(name="pts", bufs=2) as ptsp,
            tc.tile_pool(name="acc", bufs=2) as accp,
            tc.tile_pool(name="ring", bufs=2) as ringp,
            tc.tile_pool(name="t01", bufs=1) as t01p,
            tc.tile_pool(name="small", bufs=4) as smallp,
            tc.tile_pool(name="psum", bufs=PS_BUFS, space="PSUM") as psump,
        ):
            ident = constp.tile([128, 128], f16)
            make_identity(nc, ident[:])
            ones128 = constp.tile([128, 1], f32)
            nc.gpsimd.memset(ones128[:], 1.0)
            out_sb = constp.tile([1, bl], f32)

            import contextlib
            loop_ctx = (
                tc.For_i(0, repeat, 1) if repeat > 1 else contextlib.nullcontext()
            )
            with loop_ctx:
                for b in range(bl):
                    a6 = ptsp.tile([NROWS, n], f32r, tag="a6")
                    b6 = ptsp.tile([NROWS, m], f32r, tag="b6")
                    a2c = smallp.tile([128, n_stripes], f32, tag="a2c")
                    nc.sync.dma_start(a6[0:6, :], a6d.ap()[b])
                    nc.sync.dma_start(b6[0:6, :], b6d.ap()[b])
                    nc.sync.dma_start(a2c[:], a2d.ap()[b])
                    for rg in range(1, RG):
                        nc.sync.dma_start(a6[32 * rg : 32 * rg + 6, :], a6[0:6, :])
                        nc.sync.dma_start(b6[32 * rg : 32 * rg + 6, :], b6[0:6, :])

                    acc = accp.tile([128, m], f16, tag="acc")
                    # mins: cols 0:n_stripes = per-stripe row-mins,
                    #       cols n_stripes:n_stripes+m//128 = col-min blocks
                    mins = smallp.tile([128, n_stripes + m // 128], f16, tag="mins")

                    for q in range(n_quads):
                        ring = ringp.tile([128, QUAD, m], f16, tag="ring")
                        t01 = t01p.tile([128, QUAD, m // 2], f16, tag="t01")
                        for si in range(QUAD):
                            s = q * QUAD + si
                            ssl = slice(128 * s, 128 * (s + 1))
                            for g in range(n_groups):
                                ps = psump.tile([128, PSW], f32, tag="mm")
                                for j in range(PSW // 512):
                                    mo = PSW * g + 512 * j
                                    ro = 32 * ((g * (PSW // 512) + j) % RG)
                                    nc.tensor.matmul(
                                        ps[:, 512 * j : 512 * (j + 1)],
                                        a6[ro : ro + 6, ssl],
                                        b6[ro : ro + 6, mo : mo + 512],
                                        start=True,
                                        stop=True,
                                        tile_position=(ro, 0),
                                    )
                                gsl = slice(PSW * g, PSW * (g + 1))
                                if EVAC_ON:
                                    nc.scalar.activation(
                                        ring[:, si, gsl], ps[:], AF.Identity,
                                        bias=a2c[:, s : s + 1], scale=-2.0,
                                    )
                                else:
                                    nc.vector.memset(ring[:, si, gsl], 1.0)
                            # col-min accumulate, 2048-wide chunks (wider
                            # flat DVE ops hit a slow path)
                            if COLMIN_ON:
                                for c in range(m // 2048):
                                    csl = slice(2048 * c, 2048 * (c + 1))
                                    if s == 0:
                                        nc.vector.tensor_copy(
                                            acc[:, csl], ring[:, si, csl]
                                        )
                                    else:
                                        nc.vector.tensor_tensor(
                                            acc[:, csl], acc[:, csl],
                                            ring[:, si, csl], ALU.min,
                                        )
                            # per-stripe first fold: m -> m/2 (2048-out op)
                            if ROWMIN_ON:
                                nc.vector.tensor_tensor(
                                    t01[:, si, :], ring[:, si, 0 : m // 2],
                                    ring[:, si, m // 2 : m], ALU.min,
                                )
                        # quad-batched fold tree: one op per level, 4 stripes
                        if ROWMIN_ON:
                            w = m // 4
                            while w >= 128:
                                nc.vector.tensor_tensor(
                                    t01[:, :, 0:w], t01[:, :, 0:w],
                                    t01[:, :, w : 2 * w], ALU.min,
                                )
                                w //= 2
                            nc.vector.tensor_reduce(
                                mins[:, q * QUAD : (q + 1) * QUAD],
                                t01[:, :, 0:128],
                                axis=AX.X,
                                op=ALU.min,
                            )
                        else:
                            nc.vector.memset(
                                mins[:, q * QUAD : (q + 1) * QUAD], 0.0
                            )

                    # col-min across partitions: 16 transposes per psum tile,
                    # then one strided reduce-min per psum tile.
                    n_blocks = m // 128
                    if COLMIN_ON:
                        tpb = PSW // 128
                        for k0 in range(0, n_blocks, tpb):
                            pst = psump.tile([128, PSW], f16, tag="mm")
                            kk = min(tpb, n_blocks - k0)
                            for k in range(kk):
                                nc.tensor.transpose(
                                    pst[:, 128 * k : 128 * (k + 1)],
                                    acc[:, 128 * (k0 + k) : 128 * (k0 + k + 1)],
                                    ident[:],
                                )
                            nc.vector.tensor_reduce(
                                mins[:, n_stripes + k0 : n_stripes + k0 + kk],
                                pst[:, 0 : 128 * kk].rearrange(
                                    "p (k x) -> p k x", x=128
                                ),
                                axis=AX.X,
                                op=ALU.min,
                            )
                    else:
                        nc.vector.memset(mins[:, n_stripes:], 0.0)

                    # single merged sum: reduce-add all row-mins and col-mins
                    # then one ones-matmul to collapse partitions
                    tsum = smallp.tile([128, 1], f32, tag="tsum")
                    nc.vector.tensor_reduce(tsum[:], mins[:], axis=AX.X, op=ALU.add)
                    sc = psump.tile([128, PSW], f32, tag="mm")
                    nc.tensor.matmul(
                        sc[0:1, 0:1], tsum[:], ones128[:], start=True, stop=True
                    )
                    nc.vector.tensor_copy(out_sb[0:1, b : b + 1], sc[0:1, 0:1])

                nc.sync.dma_start(out.ap(), out_sb[:])

    nc.finalize()
    return nc


def _prep(points, bl):
    """Host-side: [B, N, 3] fp32 -> per-core lhsT/rhs arrays + |a|^2 bias.

    Returns (x6 [B, NROWS, N], x2c [B, 128, N//128]) where x6 rows
    32*rg + (0..5) = [x, y, z, -0.5, -0.5, -0.5] replicated for each PE
    row-group, and rhs rows 3..5 hold the squared coords instead of -0.5
    (the b-side). The caller picks which rows matter.
    """
    B, N, _ = points.shape
    xT = points.transpose(0, 2, 1)  # [B, 3, N]
    x6 = np.zeros((B, 6, N), dtype=np.float32)
    sq = xT * xT
    x2 = sq.sum(axis=1)  # [B, N]
    x6[:, 0:3] = xT
    x2c = np.ascontiguousarray(
        x2.reshape(B, N // 128, 128).transpose(0, 2, 1)
    )  # [B, 128, N//128], x2c[b, p, s] = |x_{128 s + p}|^2
    return x6, sq, x2c


def _in_maps(points1, points2):
    points1 = np.ascontiguousarray(np.asarray(points1), dtype=np.float32)
    points2 = np.ascontiguousarray(np.asarray(points2), dtype=np.float32)
    btot = points1.shape[0]
    bl = btot // N_CORES
    a6, _, a2c = _prep(points1, bl)
    b6, bsq, _ = _prep(points2, bl)
    # a-side rows 3:5 = -0.5 consts; b-side rows 3:5 = squared coords
    a6[:, 3:6] = -0.5
    b6[:, 3:6] = bsq
    return [
        {
            "a6d": a6[c * bl : (c + 1) * bl],
            "b6d": b6[c * bl : (c + 1) * bl],
            "a2d": a2c[c * bl : (c + 1) * bl],
        }
        for c in range(N_CORES)
    ]


def kernel(points1, points2):
    global last_exec_seconds
    points1 = np.ascontiguousarray(np.asarray(points1), dtype=np.float32)
    points2 = np.ascontiguousarray(np.asarray(points2), dtype=np.float32)
    btot, n, _ = points1.shape
    m = points2.shape[1]
    bl = btot // N_CORES

    key = (bl, n, m)
    if _CACHE.get("key") != key:
        _CACHE["nc"] = _build(bl, n, m)
        _CACHE["key"] = key
    nc = _CACHE["nc"]

    in_maps = _in_maps(points1, points2)
    t0 = time.time()
    res = bass_utils.run_bass_kernel_spmd(
        nc, in_maps, core_ids=list(range(N_CORES))
    )
    last_exec_seconds = time.time() - t0

    total = np.float64(0.0)
    for r in res.results:
        total += r["out"].astype(np.float64).sum()
    return np.float32(total / btot)


# revision 24
# speedup vs baseline: 1.5763x; 1.0536x over previous
"""Chamfer distance loss on 8 Trainium2 NeuronCores.

Full inputs: points1 [16, 4096, 3], points2 [16, 4096, 3] (fp32).
Output: scalar fp32 loss = (sum(min_m dist) + sum(min_n dist)) / B.

Sharding: data-parallel over batch B=16 -> 2 batches per core on 8 cores.
Each core computes a partial scalar (sum of row-mins + col-mins for its
batches); host sums the 8 partials and divides by B.

Per-batch device algorithm (per core), v3:
  dist[n, m] = |a_n|^2 + |b_m|^2 - 2 a.b  computed as:
    psum = matmul(lhsT=[ax,ay,az,-.5,-.5,-.5], rhs=[bx,by,bz,bx^2,by^2,bz^2])
         = a.b - |b|^2/2                       (K=6, fp32r, N=512 per bank)
    dist16 = ScalarE Identity((-2)*psum + bias)  bias = |a_n|^2 per partition
  All matmul operands (including the replicated row groups for PE
  tile_position concurrency) and the |a|^2 bias columns are PRECOMPUTED ON
  HOST and DMA'd in directly - no device-side staging/squaring.
  Stripes (128 rows of n) are processed in QUADS of 4; the bf16 dist tiles
  of a quad live in one ring tile [128, 4, 4096] so the row-min fold tree
  runs as ONE DVE op per level over all 4 stripes ([128, 4, w] 3D APs) -
  DVE per-op overhead dominated v1. DVE ops stay <= 2048 elems/partition
  wide (wider flat ops hit a slow path).
  col-min: DVE tensor_tensor min into acc per stripe; final col-min across
  partitions via PE transpose + strided reduce-min; row+col sums via one
  merged reduce-add + one matmul with ones.
"""

import time

import numpy as np

import concourse.bacc as bacc
import concourse.mybir as mybir
import concourse.tile as tile
from concourse import bass_utils
from concourse.masks import make_identity

N_CORES = 8

f32 = mybir.dt.float32
f32r = mybir.dt.float32r
f16 = mybir.dt.bfloat16
AF = mybir.ActivationFunctionType
ALU = mybir.AluOpType
AX = mybir.AxisListType

_CACHE = {}
last_exec_seconds = None  # wall time of the device dispatch (set per call)

QUAD = 4         # stripes per quad (ring depth)
K7 = True        # fold |a|^2 into the matmul (K=7) -> bias-free activations
PSW = 2048       # psum group width (2048 | 4096)
PS_BUFS = 2      # psum pool bufs (PSW//512 banks each; total <= 8 banks)
RG = 4           # PE row-groups for concurrent matmuls (1 | 2 | 4)
EVAC_ON = True   # timing attribution: ScalarE evacuation
ROWMIN_ON = True  # timing attribution: t01 + quad fold tree
COLMIN_ON = True  # timing attribution: colacc TTs

KDIM = 7 if K7 else 6
NROWS = 32 * (RG - 1) + KDIM


def _build(bl: int, n: int, m: int, repeat: int = 1):
    """Build the SPMD module for bl batches of [n x 3] vs [m x 3] points.

    repeat > 1 wraps the whole computation in a hardware For_i loop that
    recomputes the same result `repeat` times — used only for timing.
    """
    assert n % (128 * QUAD) == 0 and m % PSW == 0
    n_stripes = n // 128
    n_quads = n_stripes // QUAD
    n_groups = m // PSW

    nc = bacc.Bacc("TRN2", target_bir_lowering=False, debug=False)
    a6d = nc.dram_tensor("a6d", [bl, KDIM, n], f32r, kind="ExternalInput")
    b6d = nc.dram_tensor("b6d", [bl, KDIM, m], f32r, kind="ExternalInput")
    if not K7:
        a2d = nc.dram_tensor("a2d", [bl, 128, n // 128], f32, kind="ExternalInput")
    out = nc.dram_tensor("out", [1, bl], f32, kind="ExternalOutput")

    with tile.TileContext(nc) as tc:
        with (
            tc.tile_pool(name="const", bufs=1) as constp,
            tc.tile_pool(name="pts", bufs=2) as ptsp,
            tc.tile_pool(name="acc", bufs=2) as accp,
            tc.tile_pool(name="ring", bufs=2) as ringp,
            tc.tile_pool(name="t01", bufs=1) as t01p,
            tc.tile_pool(name="small", bufs=4) as smallp,
            tc.tile_pool(name="psum", bufs=PS_BUFS, space="PSUM") as psump,
        ):
            ident = constp.tile([128, 128], f16)
            make_identity(nc, ident[:])
            ones128 = constp.tile([128, 1], f32)
            nc.gpsimd.memset(ones128[:], 1.0)
            out_sb = constp.tile([1, bl], f32)

            import contextlib
            loop_ctx = (
                tc.For_i(0, repeat, 1) if repeat > 1 else contextlib.nullcontext()
            )
            with loop_ctx:
                for b in range(bl):
                    a6 = ptsp.tile([NROWS, n], f32r, tag="a6")
                    b6 = ptsp.tile([NROWS, m], f32r, tag="b6")
                    nc.sync.dma_start(a6[0:KDIM, :], a6d.ap()[b])
                    nc.sync.dma_start(b6[0:KDIM, :], b6d.ap()[b])
                    if not K7:
                        a2c = smallp.tile([128, n_stripes], f32, tag="a2c")
                        nc.sync.dma_start(a2c[:], a2d.ap()[b])
                    for rg in range(1, RG):
                        nc.sync.dma_start(
                            a6[32 * rg : 32 * rg + KDIM, :], a6[0:KDIM, :]
                        )
                        nc.sync.dma_start(
                            b6[32 * rg : 32 * rg + KDIM, :], b6[0:KDIM, :]
                        )

                    acc = accp.tile([128, m], f16, tag="acc")
                    # mins: cols 0:n_stripes = per-stripe row-mins,
                    #       cols n_stripes:n_stripes+m//128 = col-min blocks
                    mins = smallp.tile([128, n_stripes + m // 128], f16, tag="mins")

                    for q in range(n_quads):
                        ring = ringp.tile([128, QUAD, m], f16, tag="ring")
                        t01 = t01p.tile([128, QUAD, m // 2], f16, tag="t01")
                        for si in range(QUAD):
                            s = q * QUAD + si
                            ssl = slice(128 * s, 128 * (s + 1))
                            for g in range(n_groups):
                                ps = psump.tile([128, PSW], f32, tag="mm")
                                for j in range(PSW // 512):
                                    mo = PSW * g + 512 * j
                                    ro = 32 * ((g * (PSW // 512) + j) % RG)
                                    nc.tensor.matmul(
                                        ps[:, 512 * j : 512 * (j + 1)],
                                        a6[ro : ro + KDIM, ssl],
                                        b6[ro : ro + KDIM, mo : mo + 512],
                                        start=True,
                                        stop=True,
                                        tile_position=(ro, 0),
                                    )
                                gsl = slice(PSW * g, PSW * (g + 1))
                                if EVAC_ON:
                                    nc.scalar.activation(
                                        ring[:, si, gsl], ps[:], AF.Identity,
                                        bias=(
                                            0.0 if K7 else a2c[:, s : s + 1]
                                        ),
                                        scale=-2.0,
                                    )
                                else:
                                    nc.vector.memset(ring[:, si, gsl], 1.0)
                            # col-min accumulate, 2048-wide chunks (wider
                            # flat DVE ops hit a slow path)
                            if COLMIN_ON:
                                for c in range(m // 2048):
                                    csl = slice(2048 * c, 2048 * (c + 1))
                                    if s == 0:
                                        nc.vector.tensor_copy(
                                            acc[:, csl], ring[:, si, csl]
                                        )
                                    else:
                                        nc.vector.tensor_tensor(
                                            acc[:, csl], acc[:, csl],
                                            ring[:, si, csl], ALU.min,
                                        )
                            # per-stripe first fold: m -> m/2 (2048-out op)
                            if ROWMIN_ON:
                                nc.vector.tensor_tensor(
                                    t01[:, si, :], ring[:, si, 0 : m // 2],
                                    ring[:, si, m // 2 : m], ALU.min,
                                )
                        # quad-batched fold tree: one op per level, 4 stripes
                        if ROWMIN_ON:
                            w = m // 4
                            while w >= 128:
                                nc.vector.tensor_tensor(
                                    t01[:, :, 0:w], t01[:, :, 0:w],
                                    t01[:, :, w : 2 * w], ALU.min,
                                )
                                w //= 2
                            nc.vector.tensor_reduce(
                                mins[:, q * QUAD : (q + 1) * QUAD],
                                t01[:, :, 0:128],
                                axis=AX.X,
                                op=ALU.min,
                            )
                        else:
                            nc.vector.memset(
                                mins[:, q * QUAD : (q + 1) * QUAD], 0.0
                            )

                    # col-min across partitions: 16 transposes per psum tile,
                    # then one strided reduce-min per psum tile.
                    n_blocks = m // 128
                    if COLMIN_ON:
                        tpb = PSW // 128
                        for k0 in range(0, n_blocks, tpb):
                            pst = psump.tile([128, PSW], f16, tag="mm")
                            kk = min(tpb, n_blocks - k0)
                            for k in range(kk):
                                nc.tensor.transpose(
                                    pst[:, 128 * k : 128 * (k + 1)],
                                    acc[:, 128 * (k0 + k) : 128 * (k0 + k + 1)],
                                    ident[:],
                                )
                            nc.vector.tensor_reduce(
                                mins[:, n_stripes + k0 : n_stripes + k0 + kk],
                                pst[:, 0 : 128 * kk].rearrange(
                                    "p (k x) -> p k x", x=128
                                ),
                                axis=AX.X,
                                op=ALU.min,
                            )
                    else:
                        nc.vector.memset(mins[:, n_stripes:], 0.0)

                    # single merged sum: reduce-add all row-mins and col-mins
                    # then one ones-matmul to collapse partitions
                    tsum = smallp.tile([128, 1], f32, tag="tsum")
                    nc.vector.tensor_reduce(tsum[:], mins[:], axis=AX.X, op=ALU.add)
                    sc = psump.tile([128, PSW], f32, tag="mm")
                    nc.tensor.matmul(
                        sc[0:1, 0:1], tsum[:], ones128[:], start=True, stop=True
                    )
                    nc.vector.tensor_copy(out_sb[0:1, b : b + 1], sc[0:1, 0:1])

                nc.sync.dma_start(out.ap(), out_sb[:])

    nc.finalize()
    return nc


def _prep(points, bl):
    """Host-side: [B, N, 3] fp32 -> per-core lhsT/rhs arrays + |a|^2 bias.

    Returns (x6 [B, NROWS, N], x2c [B, 128, N//128]) where x6 rows
    32*rg + (0..5) = [x, y, z, -0.5, -0.5, -0.5] replicated for each PE
    row-group, and rhs rows 3..5 hold the squared coords instead of -0.5
    (the b-side). The caller picks which rows matter.
    """
    B, N, _ = points.shape
    xT = points.transpose(0, 2, 1)  # [B, 3, N]
    x6 = np.zeros((B, KDIM, N), dtype=np.float32)
    sq = xT * xT
    x2 = sq.sum(axis=1)  # [B, N]
    x6[:, 0:3] = xT
    x2c = np.ascontiguousarray(
        x2.reshape(B, N // 128, 128).transpose(0, 2, 1)
    )  # [B, 128, N//128], x2c[b, p, s] = |x_{128 s + p}|^2
    return x6, sq, x2, x2c


def _in_maps(points1, points2):
    points1 = np.ascontiguousarray(np.asarray(points1), dtype=np.float32)
    points2 = np.ascontiguousarray(np.asarray(points2), dtype=np.float32)
    btot = points1.shape[0]
    bl = btot // N_CORES
    a6, _, a2, a2c = _prep(points1, bl)
    b6, bsq, _, _ = _prep(points2, bl)
    # a-side rows 3:5 = -0.5 consts; b-side rows 3:5 = squared coords
    a6[:, 3:6] = -0.5
    b6[:, 3:6] = bsq
    if K7:
        # psum = a.b - |b|^2/2 - |a|^2/2 = -dist/2; act scale=-2, no bias
        a6[:, 6] = -0.5 * a2
        b6[:, 6] = 1.0
    maps = [
        {
            "a6d": a6[c * bl : (c + 1) * bl],
            "b6d": b6[c * bl : (c + 1) * bl],
        }
        for c in range(N_CORES)
    ]
    if not K7:
        for c in range(N_CORES):
            maps[c]["a2d"] = a2c[c * bl : (c + 1) * bl]
    return maps


def kernel(points1, points2):
    global last_exec_seconds
    points1 = np.ascontiguousarray(np.asarray(points1), dtype=np.float32)
    points2 = np.ascontiguousarray(np.asarray(points2), dtype=np.float32)
    btot, n, _ = points1.shape
    m = points2.shape[1]
    bl = btot // N_CORES

    key = (bl, n, m)
    if _CACHE.get("key") != key:
        _CACHE["nc"] = _build(bl, n, m)
        _CACHE["key"] = key
    nc = _CACHE["nc"]

    in_maps = _in_maps(points1, points2)
    t0 = time.time()
    res = bass_utils.run_bass_kernel_spmd(
        nc, in_maps, core_ids=list(range(N_CORES))
    )
    last_exec_seconds = time.time() - t0

    total = np.float64(0.0)
    for r in res.results:
        total += r["out"].astype(np.float64).sum()
    return np.float32(total / btot)
